# revision 23
# baseline (speedup 1.0000x reference)
"""Trainium2 SPMD kernel for LoFTR cross-attention (nn_LoFTRAttention).

Full inputs: x [2,2048,1024], src [2,2048,1024], Wq/Wk/Wv/Wo [1024,1024].
Reference: y = MHA(q=x@Wq.T, k=src@Wk.T, v=src@Wv.T, 16 heads of 64) @ Wo.T

Sharding over 8 NeuronCores: data-parallel on batch (2) x tensor-parallel on
heads (4 groups of 4 heads). Core c: batch c//4, heads [4*(c%4), 4*(c%4)+4).
Each core computes its heads' full attention + its slice of the output
projection (row-split Wo); the host sums the 4 partial outputs per batch
(the all-reduce of the row-split projection). Partial y is written bf16
(halves the output DMA; the host accumulates in fp32).

Device-side layout (per core):
  qT/kT [256, L] (head dims on partitions, pair-major) so the QK^T matmul
  contracts d on the partition dim and lands scores TRANSPOSED ([j, i]) --
  exactly the layout the P@V matmul needs as its moving operand. V is
  augmented with a block of ones columns so each P@V matmul also emits the
  softmax denominator (replicated on partitions 64..127) for free; softmax
  needs no max-subtraction (scores ~ N(0,1); exp stays in fp32 range).
  All matmul operands bf16 (full PE rate), fp32 PSUM accumulate.

Schedule (single Tile region; emission order = per-engine program order):
  - Input DMA is chunked (weights first, then srcT/xT in 1024-column
    blocks) and the projection chains chase the arrivals, so the PE starts
    ~12us in instead of waiting ~40us for all inputs.
  - Inside an attention unit the P@V matmul for step j is emitted AFTER
    the QK^T matmul for step j+lead: the in-order PE then always has
    independent work while the ACT engine computes exp(sT[j]).
  - Projection work (q/k chains for the next pair, final-projection
    pieces) is spread as fillers, ~1 matmul per j-step, instead of
    multi-us bursts at unit boundaries. Final-projection fillers start a
    few steps in: they depend on the previous unit's normalize.
  - Softmax normalization first evacuates the P@V accumulator PSUM->SBUF
    (one cheap DVE copy releases the bank for the next unit), then runs
    the slow exact InstReciprocal (~6.9us; the only divide available on
    this walrus build) in 128-column chunks off the critical path, with
    the multiplies on the otherwise-idle GpSimd engine.
"""

import numpy as np
import ml_dtypes

import concourse.bass as bass
import concourse.mybir as mybir
from concourse.tile import TileContext
from concourse.vector_clock import ScopedClock
from concourse.bass_utils import run_bass_kernel_spmd

F32 = mybir.dt.float32
BF16 = mybir.dt.bfloat16
AF = mybir.ActivationFunctionType
P = 128
ts, ds = bass.ts, bass.ds

B, L, D, NHEAD, DH = 2, 2048, 1024, 16, 64
N_CORES = 8
GROUPS = N_CORES // B          # head groups per batch = 4
HPC = NHEAD // GROUPS          # heads per core = 4
C = HPC * DH                   # per-core projected width = 256
SCORE_MODE = "plain"

_waitsplit_patched = False


def _patch_wait_splitting(maxw=1):
    """This walrus build caps the sem-wait count encodable on a single
    instruction ('Too many sync wait commands'). Split excess waits into
    standalone EventSemaphore instructions (same engine, directly before the
    instruction) at BIR-JSON level, right before the backend compile."""
    global _waitsplit_patched
    if _waitsplit_patched:
        return
    _waitsplit_patched = True
    import orjson
    from concourse import bass2jax, bass_utils

    orig = bass_utils.compile_bir_kernel

    def _split(bir_json):
        d = orjson.loads(bir_json)
        changed = False
        for fn in d.get("functions", []):
            for bb in fn.get("blocks") or []:
                out = []
                for ins in bb.get("instructions", []):
                    si = ins.get("sync_info")
                    waits = (si or {}).get("on_wait") or []
                    if len(waits) > maxw:
                        changed = True
                        for i, w in enumerate(waits[: len(waits) - maxw]):
                            out.append(
                                {
                                    "debug": ins.get("debug", 0),
                                    "engine": ins["engine"],
                                    "ins": [],
                                    "name": f"{ins['name']}-hw{i}",
                                    "opcode": "EventSemaphore",
                                    "outs": [],
                                    "sync_info": {"on_update": [], "on_wait": [w]},
                                }
                            )
                        si["on_wait"] = waits[len(waits) - maxw :]
                    out.append(ins)
                bb["instructions"] = out
        return orjson.dumps(d) if changed else bir_json

    def wrapped(bir_json, tmpdir, neff_name="file.neff"):
        return orig(_split(bir_json), tmpdir, neff_name)

    bass_utils.compile_bir_kernel = wrapped
    bass2jax.compile_bir_kernel = wrapped


_drain_patched = False


def _patch_tile_drain():
    """This walrus build rejects sem waits on the SP Drain instruction
    ('Too many sync wait commands'); emit explicit SP wait_ge's instead."""
    global _drain_patched
    if _drain_patched:
        return
    _drain_patched = True

    def _drain_and_barrier(self, tick_clock, wait_clock):
        nc = self.nc
        nop_inst = nc.sync.nop(nofuse=True)
        wait_clock.add_sem_waits(
            nop_inst.ins, ScopedClock({None: tick_clock.global_clock})
        )
        waits = list(nop_inst.ins.sync_info.on_wait)
        nop_inst.ins.sync_info.on_wait.clear()
        assert self.sems is not None
        num_to_handle = {h.num: h for h in self.sems.allocated().values()}
        for w in waits:
            h = num_to_handle.get(w.id)
            if h is None:
                raise RuntimeError(f"no semaphore handle for drain wait {w}")
            nc.sync.wait_ge(h, w.wait_value)
        nc.sync.drain()
        nc.all_engine_barrier()
        popped = nc._tile_sem_poison_stack.pop()
        assert popped is self._sem_poison
        nc.clear_and_free_semaphores(list(self.sems.allocated().values()))
        nc.all_engine_barrier()

    TileContext._drain_and_barrier = _drain_and_barrier


def build(score_mode="plain"):
    """Per-core Bass program (SPMD: same program, per-core data)."""
    _patch_tile_drain()
    _patch_wait_splitting()
    KO = D // P                 # contraction chunks for projections (8)
    MQ = C // P                 # head pairs (2)
    NJ = L // P                 # key chunks (16)
    NI4 = L // 512              # attention i-units (4)
    SW = 512                    # matmul slice width
    CO = C // P                 # final-projection contraction chunks (2)
    NN = D // 512               # output 512-slices (2)
    IB = 2                      # xT/srcT DMA column-blocks
    LB = L // IB                # block width (1024)
    SCALE = DH ** -0.5
    packed = score_mode == "packed"

    nc = bass.Bass()
    xT = nc.declare_dram_parameter("xT", [D, L], BF16, isOutput=False)
    srcT = nc.declare_dram_parameter("srcT", [D, L], BF16, isOutput=False)
    wqT = nc.declare_dram_parameter("wqT", [D, C], BF16, isOutput=False)
    wkT = nc.declare_dram_parameter("wkT", [D, C], BF16, isOutput=False)
    wvT = nc.declare_dram_parameter("wvT", [D, C], BF16, isOutput=False)
    woT = nc.declare_dram_parameter("woT", [C, D], BF16, isOutput=False)
    y = nc.declare_dram_parameter("y", [L, D], BF16, isOutput=True)

    with TileContext(nc) as tc:
        with (
            tc.tile_pool(name="const", bufs=1) as const_pool,
            tc.tile_pool(name="acts", bufs=1) as acts_pool,
            tc.tile_pool(name="pp_psum", bufs=2, space="PSUM") as pp_psum,
            tc.tile_pool(name="sT_psum", bufs=2, space="PSUM") as sT_psum,
            tc.tile_pool(name="out_psum", bufs=1, space="PSUM") as out_psum,
            tc.tile_pool(name="expS", bufs=5) as expS_pool,
            tc.tile_pool(name="rz", bufs=2) as rz_pool,
            tc.tile_pool(name="y_sb", bufs=4) as y_pool,
        ):
            # [P, block, ko, cols-in-block]: each DMA block is contiguous.
            xT_sb = const_pool.tile([P, IB, KO, LB], BF16)
            srcT_sb = const_pool.tile([P, IB, KO, LB], BF16)
            wq_sb = const_pool.tile([P, KO, C], BF16)
            wk_sb = const_pool.tile([P, KO, C], BF16)
            wv_sb = const_pool.tile([P, KO, C], BF16)
            wo_sb = const_pool.tile([P, CO, D], BF16)
            xT_r = xT.rearrange("(ko p) (b i) -> p b ko i", p=P, b=IB)
            srcT_r = srcT.rearrange("(ko p) (b i) -> p b ko i", p=P, b=IB)
            # Issue order = approximate arrival order: weights, then srcT
            # block 0, xT block 0, srcT block 1, xT block 1, wo.
            # Issue order approximates arrival order; the first v chains
            # need only wv + the first 512 srcT columns, so those two go
            # first. srcT block 0 lands in two 512-column halves so the
            # very first v/k chains can start a few us earlier.
            nc.sync.dma_start(wv_sb[:], wvT.rearrange("(ko p) c -> p ko c", p=P))
            nc.sync.dma_start(srcT_sb[:, 0, :, 0:512], srcT_r[:, 0, :, 0:512])
            nc.sync.dma_start(wk_sb[:], wkT.rearrange("(ko p) c -> p ko c", p=P))
            nc.sync.dma_start(srcT_sb[:, 0, :, 512:1024], srcT_r[:, 0, :, 512:1024])
            nc.sync.dma_start(wq_sb[:], wqT.rearrange("(ko p) c -> p ko c", p=P))
            nc.sync.dma_start(srcT_sb[:, 1], srcT_r[:, 1])
            nc.sync.dma_start(xT_sb[:, 0], xT_r[:, 0])
            nc.sync.dma_start(xT_sb[:, 1], xT_r[:, 1])
            nc.sync.dma_start(wo_sb[:], woT.rearrange("(co p) n -> p co n", p=P))

            qT_sb = acts_pool.tile([P, MQ, L], BF16)
            kT_sb = acts_pool.tile([P, MQ, L], BF16)
            v_sb = acts_pool.tile([P, NJ, HPC, P], BF16)  # [v_h | ones]
            outT_sb = acts_pool.tile([P, MQ, L], BF16)
            nc.vector.memset(v_sb[:, :, :, DH:], 1.0)

            def src_slice(k, i0, w):
                """srcT_sb columns [i0, i0+w) of contraction chunk k."""
                b = i0 // LB
                o = i0 - b * LB
                return srcT_sb[:, b, k, o : o + w]

            def x_slice(k, i0, w):
                b = i0 // LB
                o = i0 - b * LB
                return xT_sb[:, b, k, o : o + w]

            def v_block(b):
                """v = srcT.T @ wvT for key chunks of block b -> v_sb."""
                for j in range(b * NJ // IB, (b + 1) * NJ // IB):
                    pv = pp_psum.tile([P, HPC, DH], F32, tag="pp")
                    pvf = pv.rearrange("p h d -> p (h d)")
                    for k in range(KO):
                        nc.tensor.matmul(
                            pvf,
                            lhsT=src_slice(k, j * P, P),
                            rhs=wv_sb[:, k, :],
                            start=(k == 0),
                            stop=(k == KO - 1),
                        )
                    nc.vector.tensor_copy(v_sb[:, j, :, 0:DH], pv[:])

            def chain_pieces(w_sb, sl, dst_sb, m, i4):
                """One projection chain ([128, SW] at (m, i4), 8 accumulating
                matmuls + copy-out) as 8 single-matmul pieces."""
                st = {}

                def piece(k):
                    def go():
                        if k == 0:
                            st["t"] = pp_psum.tile(
                                [P, SW], F32, tag="pp", name=f"chain_{m}_{i4}"
                            )
                        t = st["t"]
                        nc.tensor.matmul(
                            t[:],
                            lhsT=w_sb[:, k, ts(m, P)],
                            rhs=sl(k, i4 * SW, SW),
                            start=(k == 0),
                            stop=(k == KO - 1),
                        )
                        if k == KO - 1:
                            nc.vector.tensor_copy(dst_sb[:, m, ts(i4, SW)], t[:])

                    return go

                return [piece(k) for k in range(KO)]

            def q_chain(m, i4):
                return chain_pieces(wq_sb, x_slice, qT_sb, m, i4)

            def k_chain(m, i4):
                return chain_pieces(wk_sb, src_slice, kT_sb, m, i4)

            def fp_pieces(i4):
                """Output projection for the 4 row-chunks of i4, as 8
                (2 matmuls + copy) pieces; one merged row DMA per chunk."""
                ps = []
                st = {}
                for ic in range(i4 * (SW // P), (i4 + 1) * (SW // P)):
                    for n2 in range(NN):

                        def go(ic=ic, n2=n2):
                            py = pp_psum.tile([P, 512], F32, tag="pp")
                            for c in range(CO):
                                nc.tensor.matmul(
                                    py[:],
                                    lhsT=outT_sb[:, c, ts(ic, P)],
                                    rhs=wo_sb[:, c, ts(n2, 512)],
                                    start=(c == 0),
                                    stop=(c == CO - 1),
                                )
                            if n2 == 0:
                                st[ic] = y_pool.tile(
                                    [P, D], BF16, tag="ysb", name=f"ysb_{ic}"
                                )
                            ysb = st[ic]
                            nc.vector.tensor_copy(ysb[:, ts(n2, 512)], py[:])
                            if n2 == NN - 1:
                                nc.sync.dma_start(y[ts(ic, P), :], ysb[:])

                        ps.append(go)
                return ps

            def attn_unit(pair, i4, fillers, lead=2, late_fill=False,
                          tail_fp=None):
                """One attention unit: both heads of `pair` on a 512-wide
                i-slice. sT/expS tiles are [headA 512 | headB 512]. The P@V
                matmul for step j is emitted at step j+lead so the PE never
                waits on the ACT engine; `fillers` (small projection pieces)
                are spread across the j-loop. late_fill places them in the
                second half only -- final-projection fillers depend on the
                previous unit's normalize (evac+reciprocal, ~9us latency)
                and stall the in-order PE queue if emitted too early.
                tail_fp (last unit only): its own final projection, emitted
                interleaved with a column-chunked normalize so the serial
                reciprocal is off the tail's critical path."""
                pbs = (0, DH)
                outp = out_psum.tile([P, 2 * SW], F32)  # [A | B], rows 64+: Z
                exs = {}
                nfill = len(fillers)
                fi = 0
                j0 = 9 if late_fill else 0

                def emit_pv(jj):
                    ex = exs.pop(jj)
                    for s in range(2):
                        nc.tensor.matmul(
                            outp[:, ts(s, SW)],
                            lhsT=v_sb[:, jj, 2 * pair + s, :],
                            rhs=ex[:, ts(s, SW)],
                            start=(jj == 0),
                            stop=(jj == NJ - 1),
                        )

                for j in range(NJ):
                    sT = sT_psum.tile([P, 2 * SW], F32)
                    for s, pb in enumerate(pbs):
                        kw = {"tile_position": (pb, 0)} if packed else {}
                        nc.tensor.matmul(
                            sT[:, ts(s, SW)],
                            lhsT=kT_sb[pb : pb + DH, pair, ts(j, P)],
                            rhs=qT_sb[pb : pb + DH, pair, ts(i4, SW)],
                            start=True,
                            stop=True,
                            **kw,
                        )
                    ex = expS_pool.tile([P, 2 * SW], BF16)
                    nc.scalar.activation(ex[:], sT[:], AF.Exp, scale=SCALE)
                    exs[j] = ex
                    # the first P@Vs are held two extra steps: at a unit
                    # boundary they must wait for the previous accumulator's
                    # PSUM evacuation (~2.7us visible latency), so give the
                    # PE more independent slots of cover first
                    if j == lead + 2:
                        emit_pv(0)
                        emit_pv(1)
                        emit_pv(2)
                    elif j > lead + 2:
                        emit_pv(j - lead)
                    # spread fillers over steps [j0, NJ-3]: the last steps
                    # stay filler-free so their DVE copies don't queue ahead
                    # of this unit's PSUM evacuation
                    jlast = NJ - 4
                    while (
                        fi < nfill
                        and j >= j0
                        and fi * (jlast + 1 - j0) < (min(j, jlast) + 1 - j0) * nfill
                    ):
                        fillers[fi]()
                        fi += 1
                for jj in range(NJ - lead, NJ):
                    emit_pv(jj)
                while fi < nfill:
                    fillers[fi]()
                    fi += 1
                # normalize: outT = out / Z  (Z replicated on rows 64..127).
                # First evacuate outp PSUM->SBUF with one cheap copy (~1.4us)
                # so the next unit's P@V can reclaim the PSUM bank at once;
                # the slow exact reciprocal (~6.9us DVE) then runs on the
                # SBUF copy entirely off the PSUM critical path.
                zcp = rz_pool.tile([P, 2 * SW], F32)
                zcpq = zcp.rearrange("p (s c) -> p s c", s=2)
                outpq = outp.rearrange("p (s c) -> p s c", s=2)
                if tail_fp is None:
                    nc.vector.tensor_copy(zcp[:], outp[:])
                # 128-column normalize chunks: the first chunk of outT is
                # ready ~4us after the unit ends (vs ~12us for a whole-slice
                # reciprocal), so the next unit's final-projection fillers
                # don't stall the in-order PE queue. The multiplies run on
                # the (otherwise idle) GpSimd engine, overlapping the DVE's
                # next reciprocal chunk. In the last unit the evacuation is
                # chunked as well, pulling the first tail piece ~1us earlier.
                zs = zcp[DH : 2 * DH, :].rearrange("p (s c) -> p s c", s=2)
                for q4 in range(SW // P):
                    if tail_fp is not None:
                        nc.vector.tensor_copy(
                            zcpq[:, :, ts(q4, P)], outpq[:, :, ts(q4, P)]
                        )
                    rzq = rz_pool.tile([DH, 2, P], F32, tag="rzq",
                                       name=f"rzq{q4}")
                    nc.vector.reciprocal(rzq[:], zs[:, :, ts(q4, P)])
                    for s, pb in enumerate(pbs):
                        nc.gpsimd.tensor_tensor(
                            outT_sb[pb : pb + DH, pair,
                                    i4 * SW + q4 * P : i4 * SW + (q4 + 1) * P],
                            zcp[0:DH, s * SW + q4 * P : s * SW + (q4 + 1) * P],
                            rzq[:, s, :],
                            mybir.AluOpType.mult,
                        )
                    if tail_fp is not None:
                        tail_fp[2 * q4]()
                        tail_fp[2 * q4 + 1]()

            # ---- emission schedule ----
            # Phase A: chase the input DMA blocks (srcT b0, srcT b1, xT b0).
            v_block(0)
            for f in k_chain(0, 0) + k_chain(0, 1):
                f()
            v_block(1)
            for f in k_chain(0, 2) + k_chain(0, 3):
                f()
            for f in q_chain(0, 0) + q_chain(0, 1):
                f()
            # Phase B: 8 attention units; remaining projections + the final
            # projection ride along as per-j-step fillers.
            attn_unit(0, 0, k_chain(1, 0) + k_chain(1, 1))
            attn_unit(0, 1, q_chain(0, 2) + q_chain(0, 3))
            attn_unit(0, 2, k_chain(1, 2) + k_chain(1, 3))
            attn_unit(0, 3, q_chain(1, 0) + q_chain(1, 1))
            attn_unit(1, 0, q_chain(1, 2) + q_chain(1, 3))
            attn_unit(1, 1, fp_pieces(0), late_fill=True)
            attn_unit(1, 2, fp_pieces(1), late_fill=True)
            attn_unit(1, 3, fp_pieces(2), late_fill=True,
                      tail_fp=fp_pieces(3))
    return nc


_nc_cache = {}


def get_nc(score_mode=SCORE_MODE):
    if score_mode not in _nc_cache:
        _nc_cache[score_mode] = build(score_mode)
    return _nc_cache[score_mode]


def make_in_maps(x, src, Wq, Wk, Wv, Wo):
    """Host-side sharding: slice weights per head group, transpose activations
    (the device wants contraction dims on partitions), cast to bf16."""
    bf = ml_dtypes.bfloat16
    x = np.asarray(x, np.float32)
    src = np.asarray(src, np.float32)
    WqT = np.ascontiguousarray(np.asarray(Wq, np.float32).T).astype(bf)  # [D, D]
    WkT = np.ascontiguousarray(np.asarray(Wk, np.float32).T).astype(bf)
    WvT = np.ascontiguousarray(np.asarray(Wv, np.float32).T).astype(bf)
    WoTf = np.ascontiguousarray(np.asarray(Wo, np.float32).T)            # [D, D]
    xT = [np.ascontiguousarray(x[b].T).astype(bf) for b in range(B)]
    srcT = [np.ascontiguousarray(src[b].T).astype(bf) for b in range(B)]
    in_maps = []
    for c in range(N_CORES):
        b, g = c // GROUPS, c % GROUPS
        cols = slice(C * g, C * (g + 1))
        in_maps.append(
            {
                "xT": xT[b],
                "srcT": srcT[b],
                "wqT": np.ascontiguousarray(WqT[:, cols]),
                "wkT": np.ascontiguousarray(WkT[:, cols]),
                "wvT": np.ascontiguousarray(WvT[:, cols]),
                "woT": np.ascontiguousarray(WoTf[cols, :]).astype(bf),
            }
        )
    return in_maps


def assemble(results):
    """Sum the 4 row-split partial projections per batch (host all-reduce)."""
    out = np.zeros((B, L, D), np.float32)
    for c in range(N_CORES):
        out[c // GROUPS] += np.asarray(results[c]["y"], np.float32)
    return out


def kernel(x, src, Wq, Wk, Wv, Wo):
    nc = get_nc()
    in_maps = make_in_maps(x, src, Wq, Wk, Wv, Wo)
    res = run_bass_kernel_spmd(nc, in_maps, list(range(N_CORES)))
    return assemble(res.results)
